# revision 27
# baseline (speedup 1.0000x reference)
"""Multi-head attention (B=4, N=2048, C=1024, H=16) on 8 TRN2 NeuronCores.

Sharding: (batch, head-group) pairs -> 8 cores. Core c handles batch c//2 and
heads [ (c%2)*8 , (c%2)*8+8 ). QKV weights are column-sharded per head group,
proj weights row-sharded; each core emits a partial proj output (transposed),
and the host sums the two partials per batch and adds b_proj.

v3 design (all bf16). The kernel is one software-pipelined loop over 256
rounds (p-major: head-pair p, q-block j, k-chunk kk), paced by the ACT
engine's Exp (the irreducible cost: 33.5M exps/core at 1 elem/cycle/lane
= ~220us). Per round:
  scores    : two concurrent 64x128 ROW-TILED matmuls (head 2p on PE rows
              0-63, head 2p+1 on rows 64-127 -> different row groups run
              in parallel) write S^T[128 kpos, 512 q] x 2 heads into the
              two banks of one [128,1024] psum tile (double-buffered).
  exp (ACT) : one [128,1024] Exp -> bf16 p tile, consumed immediately.
  av        : one classic 128x128 matmul per head accumulating
              [v | 1]-stationary (Z via ones column) into a 1-bank psum
              accumulator per head.
  fill      : 2 matmuls of qkv-gen / proj units trickled from a queue to
              fill the PE while ACT chews; separate 2-buffer psum ring.
After each 16-round block: softmax normalize (DVE recip + gpsimd
broadcast + DVE scale) into the ao tiles; proj units are queued per
q-block after the last head pair.
"""

import os
import sys
from collections import deque

import numpy as np

for _p in ("/root/.axon_site", "/root/.axon_site/_ro/trn_rl_repo",
           "/root/.axon_site/_ro/pypackages", "/opt/trn_rl_repo", "/opt/pypackages"):
    if os.path.isdir(_p) and _p not in sys.path:
        sys.path.append(_p)

import concourse.bacc as bacc
import concourse.mybir as mybir
import concourse.tile as tile
from concourse.bass_utils import run_bass_kernel_spmd

B, N, C = 4, 2048, 1024
H, D = 16, 64
NH = 8            # heads per core
CL = NH * D       # 512 local channels
NCORES = 8
SCALE = float(D) ** -0.5

F32 = mybir.dt.float32
BF16 = mybir.dt.bfloat16
ADD = mybir.AluOpType.add

MM_DT = "bf16"

_CACHE = {}


def build_nc(mm_dt=MM_DT):
    nc = bacc.Bacc()

    xT = nc.declare_dram_parameter("xT", [C, N], BF16, isOutput=False)
    wqk = nc.declare_dram_parameter("wqk", [C, 2 * CL], BF16, isOutput=False)
    wv = nc.declare_dram_parameter("wv", [C, CL], BF16, isOutput=False)
    wp = nc.declare_dram_parameter("wp", [CL, C], BF16, isOutput=False)
    bqk = nc.declare_dram_parameter("bqk", [128, 8], F32, isOutput=False)
    bv = nc.declare_dram_parameter("bv", [128, CL], F32, isOutput=False)
    yT = nc.declare_dram_parameter("yT", [C, N], F32, isOutput=True)

    Exp = mybir.ActivationFunctionType.Exp

    def mm(out, lhsT, rhs, start, stop):
        nc.tensor.matmul(out, lhsT, rhs, start=start, stop=stop)

    with tile.TileContext(nc) as tc:
        with (
            tc.tile_pool(name="const", bufs=1) as const,
            tc.tile_pool(name="wpool", bufs=1) as wpool,
            tc.tile_pool(name="xpool", bufs=1) as xpool,
            tc.tile_pool(name="qkpool", bufs=1) as qkpool,
            tc.tile_pool(name="vpool", bufs=1) as vpool,
            tc.tile_pool(name="aopool", bufs=1) as aopool,
            tc.tile_pool(name="ptpool", bufs=4) as ptpool,
            tc.tile_pool(name="smll", bufs=2) as smll,
            tc.tile_pool(name="ytp", bufs=3) as ytp,
            tc.tile_pool(name="PS", bufs=1, space="PSUM") as PS,
        ):
            # ---- DMAs, deadline-ordered: x j0 first, then weights ----
            bqk_t = const.tile([128, 8], F32, tag="bqk")
            nc.sync.dma_start(out=bqk_t[:], in_=bqk[:])
            bv_t = const.tile([128, CL], F32, tag="bv")
            nc.sync.dma_start(out=bv_t[:], in_=bv[:])

            xT_t = [xpool.tile([128, N], BF16, tag=f"xT{cc}", name=f"xT{cc}")
                    for cc in range(8)]
            wqk_t = []
            wv_t = []
            wp_t = []
            for cc in range(8):
                jsl = slice(0, 512)
                nc.sync.dma_start(out=xT_t[cc][:, jsl],
                                  in_=xT[cc * 128:(cc + 1) * 128, jsl])
            for cc in range(8):
                wt = wpool.tile([128, 2 * CL], BF16, tag=f"wqk{cc}", name=f"wqk{cc}")
                nc.sync.dma_start(out=wt[:], in_=wqk[cc * 128:(cc + 1) * 128, :])
                wqk_t.append(wt)
            for cc in range(8):
                vt = wpool.tile([128, CL], BF16, tag=f"wv{cc}", name=f"wv{cc}")
                nc.sync.dma_start(out=vt[:], in_=wv[cc * 128:(cc + 1) * 128, :])
                wv_t.append(vt)
            for j in range(1, 4):
                jsl = slice(j * 512, (j + 1) * 512)
                for cc in range(8):
                    nc.sync.dma_start(out=xT_t[cc][:, jsl],
                                      in_=xT[cc * 128:(cc + 1) * 128, jsl])
            for cl in range(4):
                wt = wpool.tile([128, C], BF16, tag=f"wp{cl}", name=f"wp{cl}")
                nc.sync.dma_start(out=wt[:], in_=wp[cl * 128:(cl + 1) * 128, :])
                wp_t.append(wt)

            # ---- persistent intermediates ----
            qk_t = [qkpool.tile([128, N], BF16, tag=f"qk{m}", name=f"qk{m}")
                    for m in range(8)]      # 0-3: q (pre-scaled), 4-7: k
            v_t = [vpool.tile([128, NH * 128], BF16, tag=f"v{i}", name=f"v{i}")
                   for i in range(16)]      # [v | 1 | pad] per head
            ao_t = [aopool.tile([128, N], BF16, tag=f"ao{cl}", name=f"ao{cl}")
                    for cl in range(4)]     # attention out^T (normalized)

            # v pad/ones init (once; value cols rewritten by v units)
            for i in range(16):
                v3 = v_t[i].rearrange("p (h e) -> p h e", h=NH)
                nc.gpsimd.memset(v3[:, :, 64:128], 0.0)
                nc.gpsimd.memset(v3[:, :, 64:65], 1.0)

            # ---- fill units (classic 128-contraction, 1-bank psum) ----
            # emitted: emission-order guard. A consumer instruction may only
            # be EMITTED after the unit writing its operand has been fully
            # emitted — otherwise the tile framework sees a read of
            # never-written memory and adds no RAW dependency (garbage).
            emitted = set()

            def gen_qk_unit(m, j):
                js = slice(j * 512, (j + 1) * 512)
                ms = slice(m * 128, (m + 1) * 128)
                ps = PS.tile([128, 512], F32, tag="fx", name=f"qku{m}_{j}",
                             bufs=2)
                for cc in range(8):
                    mm(ps[:], wqk_t[cc][:, ms], xT_t[cc][:, js],
                       cc == 0, cc == 7)
                    yield
                nc.vector.tensor_scalar(
                    qk_t[m][:, js], ps[:], bqk_t[:, m:m + 1], None, ADD)
                emitted.add(("qk", m, j))

            def gen_v_unit(kc):
                ks = slice(kc * 128, (kc + 1) * 128)
                ps = PS.tile([128, 512], F32, tag="fx", name=f"vu{kc}", bufs=2)
                for cc in range(8):
                    mm(ps[:], xT_t[cc][:, ks], wv_t[cc][:], cc == 0, cc == 7)
                    yield
                v3 = v_t[kc].rearrange("p (h e) -> p h e", h=NH)
                nc.vector.tensor_add(
                    v3[:, :, 0:64],
                    ps.rearrange("p (h e) -> p h e", e=64),
                    bv_t.rearrange("p (h e) -> p h e", e=64))
                emitted.add(("v", kc))

            def gen_proj_unit(j, m2):
                js = slice(j * 512, (j + 1) * 512)
                ms = slice(m2 * 128, (m2 + 1) * 128)
                ps = PS.tile([128, 512], F32, tag="fx", name=f"pu{j}_{m2}",
                             bufs=2)
                for cl in range(4):
                    mm(ps[:], wp_t[cl][:, ms], ao_t[cl][:, js],
                       cl == 0, cl == 3)
                    yield
                yt = ytp.tile([128, 512], F32, tag="yt", name="yt")
                nc.vector.tensor_copy(yt[:], ps[:])
                nc.sync.dma_start(out=yT[ms, js], in_=yt[:])

            def normalize(h, av, j):
                js = slice(j * 512, (j + 1) * 512)
                z = smll.tile([1, 512], F32, tag="z", name="z")
                nc.vector.tensor_copy(z[:], av[64:65, :])
                r = smll.tile([1, 512], F32, tag="r", name="r")
                nc.vector.reciprocal_approx_fast(out=r[:], in_=z[:])
                rb = smll.tile([64, 512], F32, tag="rb", name="rb")
                nc.gpsimd.partition_broadcast(rb[:], r[:])
                cl, po = h // 2, (h % 2) * 64
                nc.vector.tensor_mul(ao_t[cl][po:po + 64, js],
                                     av[0:64, :], rb[:])

            def drain(q, nsteps):
                done = 0
                while q and done < nsteps:
                    try:
                        next(q[0])
                        done += 1
                    except StopIteration:
                        q.popleft()
                return done

            def exhaust(g):
                for _ in g:
                    pass

            def require(q, key):
                while key not in emitted:
                    if not q:
                        raise RuntimeError(f"missing producer for {key}")
                    drain(q, 16)

            # ---- ramp: k(m4) all j + q(m0) j0 ----
            for g in (gen_qk_unit(4, 0), gen_qk_unit(4, 1), gen_qk_unit(4, 2),
                      gen_qk_unit(4, 3), gen_qk_unit(0, 0)):
                exhaust(g)

            fillq = deque()
            for kc in range(16):
                fillq.append(gen_v_unit(kc))
            for j in range(1, 4):
                fillq.append(gen_qk_unit(0, j))
            for m, base in ((5, 1), (6, 2), (7, 3)):
                for j in range(4):
                    fillq.append(gen_qk_unit(m, j))
                for j in range(4):
                    fillq.append(gen_qk_unit(base, j))

            # ---- main pipeline: 256 rounds ----
            for p in range(4):
                for j in range(4):
                    for jj in range(4):
                        require(fillq, ("qk", 4 + p, jj))
                    require(fillq, ("qk", p, j))
                    js = slice(j * 512, (j + 1) * 512)
                    h0 = 2 * p
                    h0c = h0 * 128
                    h1c = (h0 + 1) * 128
                    av0 = PS.tile([128, 512], F32, tag="av0",
                                  name=f"av0_{p}_{j}")
                    av1 = PS.tile([128, 512], F32, tag="av1",
                                  name=f"av1_{p}_{j}")
                    for kk in range(16):
                        drain(fillq, 2)
                        require(fillq, ("v", kk))
                        ks = slice(kk * 128, (kk + 1) * 128)
                        ss = PS.tile([128, 1024], F32, tag="ss", name="ss",
                                     bufs=2)
                        mm(ss[:, 0:512], qk_t[4 + p][0:64, ks],
                           qk_t[p][0:64, js], True, True)
                        mm(ss[:, 512:1024], qk_t[4 + p][64:128, ks],
                           qk_t[p][64:128, js], True, True)
                        pt = ptpool.tile([128, 1024], BF16, tag="pt", name="pt")
                        nc.scalar.activation(pt[:], ss[:], Exp)
                        mm(av0[0:65, :], v_t[kk][:, h0c:h0c + 65],
                           pt[:, 0:512], kk == 0, kk == 15)
                        mm(av1[0:65, :], v_t[kk][:, h1c:h1c + 65],
                           pt[:, 512:1024], kk == 0, kk == 15)
                    normalize(h0, av0, j)
                    normalize(h0 + 1, av1, j)
                    if p == 3:
                        for m2 in range(8):
                            fillq.append(gen_proj_unit(j, m2))

            # ---- tail ----
            while fillq:
                if drain(fillq, 4) == 0 and fillq:
                    raise RuntimeError("emission stuck")

    nc.compile()
    return nc


def make_in_maps(x, w_qkv, b_qkv, w_proj, mm_dt=MM_DT):
    np_dt = mybir.dt.np(BF16)
    x = np.asarray(x, np.float32)
    w_qkv = np.asarray(w_qkv, np.float32)
    b_qkv = np.asarray(b_qkv, np.float32)
    w_proj = np.asarray(w_proj, np.float32)
    in_maps = []
    for c in range(NCORES):
        b, g = divmod(c, 2)
        h0 = g * NH
        qs = slice(h0 * D, h0 * D + CL)
        ks = slice(C + h0 * D, C + h0 * D + CL)
        vs = slice(2 * C + h0 * D, 2 * C + h0 * D + CL)
        # fold the attention scale into q weights and bias
        wqk = np.concatenate([w_qkv[:, qs] * SCALE, w_qkv[:, ks]], axis=1)
        bq = b_qkv[qs] * SCALE
        bk = b_qkv[ks]
        bqk = np.concatenate([bq, bk]).reshape(8, 128).T  # [128, 8] col-chunks
        bv = np.broadcast_to(b_qkv[vs][None, :], (128, CL))
        in_maps.append({
            "xT": np.ascontiguousarray(x[b].T).astype(np_dt),
            "wqk": np.ascontiguousarray(wqk).astype(np_dt),
            "wv": np.ascontiguousarray(w_qkv[:, vs]).astype(np_dt),
            "wp": np.ascontiguousarray(w_proj[h0 * D:h0 * D + CL, :]).astype(np_dt),
            "bqk": np.ascontiguousarray(bqk, np.float32),
            "bv": np.ascontiguousarray(bv, np.float32),
        })
    return in_maps


def run(x, w_qkv, b_qkv, w_proj, b_proj, mm_dt=MM_DT, **spmd_kwargs):
    if mm_dt not in _CACHE:
        _CACHE[mm_dt] = build_nc(mm_dt)
    nc = _CACHE[mm_dt]
    in_maps = make_in_maps(x, w_qkv, b_qkv, w_proj, mm_dt)
    res = run_bass_kernel_spmd(nc, in_maps, core_ids=list(range(NCORES)),
                               **spmd_kwargs)
    b_proj = np.asarray(b_proj, np.float32)
    out = np.empty((B, N, C), np.float32)
    for b in range(B):
        acc = res.results[2 * b]["yT"] + res.results[2 * b + 1]["yT"]
        out[b] = acc.T + b_proj[None, :]
    return out, res


def kernel(x, w_qkv, b_qkv, w_proj, b_proj):
    out, _ = run(x, w_qkv, b_qkv, w_proj, b_proj)
    return out
